# revision 25
# baseline (speedup 1.0000x reference)
"""ChannelAttention Trainium2 kernel (Bass/Tile), data-parallel over batch.

Problem shapes (hardcoded):
  x      [8, 4096, 768] fp32
  w_qkv  [2304, 768]    fp32
  w_proj [768, 768]     fp32
  b_proj [768]          fp32
  out    [8, 4096, 768] fp32

Reference (per batch b, 8 groups of 96 channels):
  qkv = x @ w_qkv.T ; q *= N**-0.5
  attn_g = softmax(q_g.T @ k_g, axis=-1)     # [96, 96], contracts over N
  out_g  = attn_g @ v_g.T                    # [96, N]
  y = out @ w_proj.T + b_proj

Sharding: batch b -> core b (8 cores SPMD, no collectives).

Algebraic restructure (v5): channel attention collapses around two small
matrices --
  G = X^T X                      [768, 768]   (Gram, symmetric)
  attn_g = softmax(Wq_s G Wk^T)  (per group, [96, 96])
  M = Wv^T BD(attn)^T WprojT     [768, 768]
  y = x @ M + b_proj
v5 layout strategy (vs v4):
  - host supplies BOTH x (natural) and x^T; no PE transposes of x.
  - Gram accumulated PSUM-resident across all 32 token tiles (exactly
    8 banks for the upper-triangle row pieces); single evacuation pass
    casts straight to fp16. Lower blocks mirrored by 15 PE transposes.
  - phase 3 computes y^T = M^T x^T with M chunks stationary (36 loads
    total) streaming x^T in 512-col blocks; y^T written out in fp16 and
    transposed/upcast on host.
  - HAM warm-up matmuls run during the initial DMA wait; weight / x^T
    loads are queued behind the x token stream on the same engines.
All matmul operands fp16 (full PE rate), fp32 accumulation in PSUM;
softmax in fp32.
"""

import numpy as np

B, N, C = 8, 4096, 768
G = 8
GC = C // G          # 96
NCORES = 8
NT = N // 128        # 32 token tiles
CC = C // 128        # 6 chunks of the channel dim
QSCALE = float(N) ** -0.5  # 1/64

_CACHE = {}

# Gram upper-triangle row pieces: (a, src_off, width) with width <= 512.
# One PSUM bank each; 8 pieces total cover cols [128a, 768) per row a.
GRAM_PIECES = [
    (0, 0, 512),
    (1, 128, 512),
    (2, 256, 512),
    (3, 384, 384),
    (4, 512, 256),
    (5, 640, 128),
    (0, 512, 256),
    (1, 640, 128),
]


def _build_nc():
    import concourse.bass as bass
    import concourse.mybir as mybir
    import concourse.tile as tile
    from concourse import bacc

    fp16 = mybir.dt.float16
    fp32 = mybir.dt.float32

    nc = bacc.Bacc(
        "TRN2", target_bir_lowering=False, debug=False, num_devices=NCORES
    )

    xh = nc.dram_tensor("xh", [N, C], fp16, kind="ExternalInput").ap()
    xhT = nc.dram_tensor("xhT", [C, N], fp16, kind="ExternalInput").ap()
    # q/k halves of w_qkv, transposed to [c, 2*768], q pre-scaled
    wqkT = nc.dram_tensor("wqkT", [C, 2 * C], fp16, kind="ExternalInput").ap()
    # v rows of w_qkv in natural [d, a] layout
    wv = nc.dram_tensor("wv", [C, C], fp16, kind="ExternalInput").ap()
    wprojT = nc.dram_tensor("wprojT", [C, C], fp16, kind="ExternalInput").ap()
    bproj = nc.dram_tensor("bproj", [C], fp32, kind="ExternalInput").ap()
    id16d = nc.dram_tensor("id16", [128, 128], fp16, kind="ExternalInput").ap()
    yT = nc.dram_tensor("yT", [C, N], fp16, kind="ExternalOutput").ap()

    with tile.TileContext(nc) as tc:
        from contextlib import ExitStack

        with ExitStack() as ctx:
            weights = ctx.enter_context(tc.tile_pool(name="weights", bufs=1))
            persist = ctx.enter_context(tc.tile_pool(name="persist", bufs=1))
            xn_pool = ctx.enter_context(tc.tile_pool(name="xn", bufs=32))
            sm_pool = ctx.enter_context(tc.tile_pool(name="sm", bufs=4))
            ysb_pool = ctx.enter_context(tc.tile_pool(name="ysb", bufs=8))

            # ---- static SBUF tiles ----
            ident16 = weights.tile([128, 128], fp16, name="ident16")
            warm = weights.tile([128, 512], fp16, name="warm")
            wqk_sb = [
                weights.tile([128, 2 * C], fp16, name=f"wqk_{a}")
                for a in range(CC)
            ]
            wv_sb = [
                weights.tile([128, C], fp16, name=f"wv_{dd}") for dd in range(CC)
            ]
            wpg_sb = [
                weights.tile([GC, C], fp16, name=f"wpg_{g}") for g in range(G)
            ]
            bias_sb = weights.tile([128, CC], fp32, name="bias_sb")

            xT6 = [
                persist.tile([128, N], fp16, name=f"xT_{a}") for a in range(CC)
            ]
            G16 = [
                persist.tile([128, C], fp16, name=f"G16_{a}") for a in range(CC)
            ]
            M1_sb = [
                persist.tile([128, C], fp16, name=f"m1_{a}") for a in range(CC)
            ]
            e16 = [
                persist.tile([GC, GC], fp16, name=f"e16_{g}") for g in range(G)
            ]
            P6 = [persist.tile([128, C], fp16, name=f"P_{dd}") for dd in range(CC)]
            M_sb = [
                persist.tile([128, C], fp16, name=f"M_{ab}") for ab in range(CC)
            ]

            # small loads on gpsimd (SWDGE) so the HW queues start with x
            nc.gpsimd.dma_start(out=ident16, in_=id16d)
            for co in range(CC):
                nc.gpsimd.dma_start(
                    out=bias_sb[:, co : co + 1],
                    in_=bproj[co * 128 : (co + 1) * 128],
                )
            nc.vector.memset(warm, 0.03125)

            # ---- single PSUM pool: 8 bank tiles manually scheduled
            # across all phases (no pool-transition barriers) ----
            with tc.tile_pool(name="ps", bufs=1, space="PSUM") as ps:
                bank = [
                    ps.tile([128, 512], fp32, name=f"bank_{i}")
                    for i in range(8)
                ]

                # HAM warm-up: junk matmuls into bank 0 while first x tile
                # is still in flight (start+stop so no group stays open;
                # the real Gram start=True wipes the bank).
                for i in range(5):
                    woff = 128 * (i % 2)
                    nc.tensor.matmul(
                        bank[0],
                        warm[:, woff : woff + 128],
                        warm,
                        start=True,
                        stop=True,
                    )

                # ---- phase 1: Gram, PSUM-resident across all 32 tiles ----
                for t in range(NT):
                    xt = xn_pool.tile([128, C], fp16, tag="xn", name=f"xn_{t}")
                    dma_eng = nc.sync if t % 2 == 0 else nc.scalar
                    dma_eng.dma_start(out=xt, in_=xh[t * 128 : (t + 1) * 128, :])
                    # last tile: emit pieces in evacuation order so the
                    # early evacs (and M1[5]) unblock before the tail ends
                    piece_order = (
                        [4, 5, 6, 7, 2, 3, 0, 1] if t == NT - 1
                        else range(len(GRAM_PIECES))
                    )
                    for i in piece_order:
                        a, soff, w = GRAM_PIECES[i]
                        nc.tensor.matmul(
                            bank[i][:, :w],
                            xt[:, a * 128 : (a + 1) * 128],
                            xt[:, soff : soff + w],
                            start=(t == 0),
                            stop=(t == NT - 1),
                        )

                # weight + xT loads queued behind the token stream: the
                # wait_until gate keeps the scheduler from enqueueing these
                # descriptors ahead of x (x must own HBM during Gram).
                with tc.tile_wait_until(0.018):
                    for a in range(CC):
                        nc.scalar.dma_start(
                            out=wqk_sb[a], in_=wqkT[a * 128 : (a + 1) * 128, :]
                        )
                    for dd in range(CC):
                        nc.sync.dma_start(
                            out=wv_sb[dd], in_=wv[dd * 128 : (dd + 1) * 128, :]
                        )
                    for g in range(G):
                        (nc.scalar if g % 2 == 0 else nc.sync).dma_start(
                            out=wpg_sb[g], in_=wprojT[g * GC : (g + 1) * GC, :]
                        )
                # xT lands after M1 (the matmul stream measurably slows when
                # these 8KB-row descriptors write SBUF under it); still well
                # before phase 3 needs it.
                with tc.tile_wait_until(0.034):
                    for a in range(CC):
                        (nc.scalar if a % 2 == 0 else nc.sync).dma_start(
                            out=xT6[a], in_=xhT[a * 128 : (a + 1) * 128, :]
                        )

                # evacuate Gram -> fp16 G16. Pieces needed by M1[5] and the
                # first mirrors go first so the tensor engine unblocks fast.
                evac_order = [4, 5, 6, 7, 2, 3, 0, 1]
                for k, i in enumerate(evac_order):
                    a, soff, w = GRAM_PIECES[i]
                    dst = G16[a][:, soff : soff + w]
                    if k % 2 == 0:
                        nc.scalar.copy(out=dst, in_=bank[i][:, :w])
                    else:
                        nc.vector.tensor_copy(dst, bank[i][:, :w])

                # ---- phase 2a: M1 = G Wk^T descending a, mirrors for
                # M1[a] (G16[b][:, a] = G16[a][:, b]^T, b > a) emitted just
                # ahead so they pipeline behind M1[a+1] ----
                M1_BANKS = {5: (4, 5), 4: (6, 7), 3: (2, 3), 2: (0, 1),
                            1: (4, 5), 0: (6, 7)}
                MIR_BANKS = {4: [0], 3: [0, 1], 2: [4, 5, 6],
                             1: [6, 7, 2, 3], 0: [4, 5, 2, 3, 0]}
                mir_k = 0
                for a in range(CC - 1, -1, -1):
                    # mirrors needed by M1[a]: G16[b][:, a] = G16[a][:, b]^T
                    for b_, mb in zip(range(a + 1, CC), MIR_BANKS.get(a, [])):
                        mps = bank[mb][:, :64].bitcast(fp16)
                        nc.tensor.transpose(
                            mps, G16[a][:, b_ * 128 : (b_ + 1) * 128], ident16
                        )
                        dst = G16[b_][:, a * 128 : (a + 1) * 128]
                        if mir_k % 2 == 0:
                            nc.vector.tensor_copy(dst, mps)
                        else:
                            nc.scalar.copy(out=dst, in_=mps)
                        mir_k += 1

                    ba, bb = M1_BANKS[a]
                    m1a = bank[ba]
                    m1b = bank[bb][:, :256]
                    for b_ in range(CC):
                        nc.tensor.matmul(
                            m1a,
                            G16[b_][:, a * 128 : (a + 1) * 128],
                            wqk_sb[b_][:, C : C + 512],
                            start=(b_ == 0),
                            stop=(b_ == CC - 1),
                        )
                    for b_ in range(CC):
                        nc.tensor.matmul(
                            m1b,
                            G16[b_][:, a * 128 : (a + 1) * 128],
                            wqk_sb[b_][:, C + 512 : 2 * C],
                            start=(b_ == 0),
                            stop=(b_ == CC - 1),
                        )
                    if a % 2 == 0:
                        nc.scalar.copy(out=M1_sb[a][:, 0:512], in_=m1a)
                        nc.vector.tensor_copy(M1_sb[a][:, 512:C], m1b)
                    else:
                        nc.vector.tensor_copy(M1_sb[a][:, 0:512], m1a)
                        nc.scalar.copy(out=M1_sb[a][:, 512:C], in_=m1b)

                # ---- phase 2b: A_g = Wq_g M1_g -> softmax -> e16 ----
                for g in range(G):
                    a_ps = bank[g % 4][:GC, :GC]
                    for a in range(CC):
                        nc.tensor.matmul(
                            a_ps,
                            wqk_sb[a][:, g * GC : (g + 1) * GC],
                            M1_sb[a][:, g * GC : (g + 1) * GC],
                            start=(a == 0),
                            stop=(a == CC - 1),
                        )
                    nm = sm_pool.tile([GC, 1], fp32, tag="nm", name=f"nm_{g}")
                    nc.vector.tensor_reduce(
                        out=nm,
                        in_=a_ps,
                        axis=mybir.AxisListType.X,
                        op=mybir.AluOpType.max,
                        negate=True,
                    )
                    e_t = sm_pool.tile([GC, GC], fp32, tag="e", name=f"e_{g}")
                    ssum = sm_pool.tile(
                        [GC, 1], fp32, tag="ssum", name=f"ssum_{g}"
                    )
                    nc.scalar.activation(
                        e_t,
                        a_ps,
                        mybir.ActivationFunctionType.Exp,
                        bias=nm,
                        scale=1.0,
                        accum_out=ssum,
                    )
                    rs = sm_pool.tile([GC, 1], fp32, tag="rs", name=f"rs_{g}")
                    nc.vector.reciprocal(rs, ssum)
                    nc.vector.tensor_scalar_mul(e16[g], e_t, rs)

                # ---- phase 2c: P = BD(attn)^T WprojT in 128-aligned
                # d-chunks (piece matmuls land at their global-d psum
                # partitions via tile_position col offsets) ----
                def d_pieces(dd):
                    raw = []
                    for g in range(G):
                        lo, hi = g * GC, (g + 1) * GC
                        r0 = max(0, 128 * dd - lo)
                        r1 = min(GC, 128 * (dd + 1) - lo)
                        if r0 < r1:
                            raw.append((g, r0, r1, lo + r0 - 128 * dd))
                    # split pieces that violate PE col-group placement rules
                    # (M<=32 at {0,32,64,96}; M<=64 at {0,64}; M>64 only at 0)
                    out = []
                    for (g, r0, r1, p0) in raw:
                        while r0 < r1:
                            m = r1 - r0
                            if p0 == 0 or (m <= 32) or (m <= 64 and p0 == 64):
                                out.append((g, r0, r1, p0))
                                break
                            step = 32 if p0 % 64 else 64
                            step = min(step, m)
                            out.append((g, r0, r0 + step, p0))
                            r0 += step
                            p0 += step
                    return out

                for dd in range(CC):
                    for half in range(2):
                        hsl = slice(half * 384, (half + 1) * 384)
                        p_ps = bank[4 + (2 * dd + half) % 4][:, :384]
                        for (g, r0, r1, p0) in d_pieces(dd):
                            nc.tensor.matmul(
                                p_ps[p0 : p0 + (r1 - r0), :],
                                e16[g][:, r0:r1],
                                wpg_sb[g][:, hsl],
                                start=True,
                                stop=True,
                                tile_position=(0, p0) if p0 else None,
                            )
                        if (dd + half) % 2 == 0:
                            nc.scalar.copy(out=P6[dd][:, hsl], in_=p_ps)
                        else:
                            nc.vector.tensor_copy(P6[dd][:, hsl], p_ps)

                # ---- phase 2d: M = Wv^T P, rows chunk ab ----
                for ab in range(CC):
                    ba, bb = (0, 1) if ab % 2 == 0 else (2, 3)
                    m2a = bank[ba]
                    m2b = bank[bb][:, :256]
                    for dd in range(CC):
                        nc.tensor.matmul(
                            m2a,
                            wv_sb[dd][:, ab * 128 : (ab + 1) * 128],
                            P6[dd][:, 0:512],
                            start=(dd == 0),
                            stop=(dd == CC - 1),
                        )
                    for dd in range(CC):
                        nc.tensor.matmul(
                            m2b,
                            wv_sb[dd][:, ab * 128 : (ab + 1) * 128],
                            P6[dd][:, 512:C],
                            start=(dd == 0),
                            stop=(dd == CC - 1),
                        )
                    if ab % 2 == 0:
                        nc.scalar.copy(out=M_sb[ab][:, 0:512], in_=m2a)
                        nc.vector.tensor_copy(M_sb[ab][:, 512:C], m2b)
                    else:
                        nc.vector.tensor_copy(M_sb[ab][:, 0:512], m2a)
                        nc.scalar.copy(out=M_sb[ab][:, 512:C], in_=m2b)

                # ---- phase 3: y^T = M^T x^T + b (M stationary, x^T
                # streams); banks 4-7 / 0-3 ping-pong per co-half ----
                for co in range(CC):
                    for half in range(2):
                        tbs = range(half * 4, half * 4 + 4)
                        yps = [bank[(4 - 4 * half) + i] for i in range(4)]
                        for i, tb in enumerate(tbs):
                            for a in range(CC):
                                nc.tensor.matmul(
                                    yps[i],
                                    M_sb[a][:, co * 128 : (co + 1) * 128],
                                    xT6[a][:, tb * 512 : (tb + 1) * 512],
                                    start=(a == 0),
                                    stop=(a == CC - 1),
                                )
                        for i, tb in enumerate(tbs):
                            ysb = ysb_pool.tile(
                                [128, 512], fp16, tag="ysb",
                                name=f"ysb_{co}_{tb}",
                            )
                            if tb % 2 == 0:
                                nc.scalar.add(
                                    ysb, yps[i], bias_sb[:, co : co + 1]
                                )
                            else:
                                nc.vector.tensor_scalar_add(
                                    ysb, yps[i], bias_sb[:, co : co + 1]
                                )
                            (nc.sync if tb % 2 == 0 else nc.scalar).dma_start(
                                out=yT[
                                    co * 128 : (co + 1) * 128,
                                    tb * 512 : (tb + 1) * 512,
                                ],
                                in_=ysb,
                            )

    nc.compile()
    return nc


def _get_nc():
    if "nc" not in _CACHE:
        _CACHE["nc"] = _build_nc()
    return _CACHE["nc"]


def _host_prep(x, w_qkv, w_proj, b_proj):
    x = np.asarray(x, dtype=np.float32)
    w_qkv = np.asarray(w_qkv, dtype=np.float32)
    w_proj = np.asarray(w_proj, dtype=np.float32)
    b_proj = np.asarray(b_proj, dtype=np.float32)

    wqk = w_qkv[: 2 * C, :].copy()
    wqk[:C, :] *= np.float32(QSCALE)
    wqkT_h = np.ascontiguousarray(wqk.T).astype(np.float16)       # [768, 1536]
    wv_h = np.ascontiguousarray(w_qkv[2 * C :, :]).astype(np.float16)
    wprojT_h = np.ascontiguousarray(w_proj.T).astype(np.float16)  # [768, 768]

    id16 = np.eye(128, dtype=np.float16)
    in_maps = []
    for b_ in range(NCORES):
        xb16 = np.ascontiguousarray(x[b_]).astype(np.float16)
        in_maps.append(
            {
                "xh": xb16,
                "xhT": np.ascontiguousarray(xb16.T),
                "wqkT": wqkT_h,
                "wv": wv_h,
                "wprojT": wprojT_h,
                "bproj": b_proj,
                "id16": id16,
            }
        )
    return in_maps


def _run(in_maps, trace=False):
    from concourse.bass_utils import run_bass_kernel_spmd

    nc = _get_nc()
    res = run_bass_kernel_spmd(nc, in_maps, list(range(NCORES)), trace=trace)
    out = np.stack(
        [
            np.ascontiguousarray(res.results[i]["yT"].T).astype(np.float32)
            for i in range(NCORES)
        ],
        axis=0,
    )
    return out, res


def kernel(x, w_qkv, w_proj, b_proj):
    in_maps = _host_prep(x, w_qkv, w_proj, b_proj)
    out, _ = _run(in_maps, trace=False)
    return out


def run_profiled(x, w_qkv, w_proj, b_proj):
    """Returns (out, BassKernelResults) with NTFF profiling enabled."""
    in_maps = _host_prep(x, w_qkv, w_proj, b_proj)
    return _run(in_maps, trace=True)
